# revision 22
# baseline (speedup 1.0000x reference)
"""Gated linear attention (GLA) Bass kernel for Trainium2, 8 NeuronCores.

Sharding: one core per (batch, head) pair -- B=2 x H=4 = 8 cores.
Each core computes its head's full pipeline with a chunked-parallel form of
the gated recurrence (chunk = 128), entirely on-device.

v3 structure (per 512-wide time slice j):
  gate:  pg = [Wgk|Wgk]^T x^T (duplicated M=128) -> sp = softplus ->
         per-chunk cumsum (all on [128,512] with both halves identical)
         -> eeqk = exp(scv*spc + bv) one ACT op (scv/bv per-partition
         vectors: q-half -G/16 + ln(scale), k-half +G/16)
         -> dlast = exp(-spc_last/16) on all 128 rows
  qk:    pqk = [Wq|Wk]^T x^T -> qkt = pqk * eeqk (one DVE op, bf16)
         kh[64:] = qkt[64:] * dlast ; ktn = kh^T, ONE batched DMA-transpose
  vg:    pv = Wv^T x^T, pgt = Wg^T x^T (transposed [dv,512])
         vb = v natural via ONE batched DMA-transpose of the pv eviction
         swT = pgt * 1/(1+exp(-pgt)) (ACT exp + DVE add/recip/mul)
  chunks (bf16 matmuls):
         pat[s,t] = kt_c^T qt_c ; atm = mask(pat) (batched per slice)
         poT = Wv-side: poT[e,t] = vb_c^T atm + S^T qt_c  (transposed O!)
         pds = ktn^T vb (pre-scaled by dlast via kh)
         S' = S*dlast + pds  (single fused scalar_tensor_tensor)
  out (no transposes):
         onbT = bf16(poT); sqT = poT^2 ; msT[1,512] = ones^T sqT (matmul)
         rstdT = exp(-.5 ln(ms/dv+eps)); bc[10,512] = ones1^T rstdT (matmul)
         og = onbT * swT ; p10 = wf^T og ; out = p10 * bc (rstd folded in)

Host does weight-only preprocessing (Wgk1@Wgk2, gnorm*Wo@Whead) and the
final head-sum + bhead add.
"""
import sys, os
sys.path.insert(0, "/opt/trn_rl_repo")

import numpy as np
import ml_dtypes

B, T, D = 2, 2048, 512
H = 4
dk, dv = 64, 128          # per-head key/value dims
C = 128                   # chunk length
GATE_NORM = 16.0
EPS = 1e-5
SCALE = dk ** -0.5

_CACHE = {}


def build(t=T):
    import concourse.bass as bass  # noqa: F401
    from concourse import bacc, mybir
    import concourse.tile as tile
    import concourse.hw_specs as hw_specs

    F32 = mybir.dt.float32
    F32R = mybir.dt.float32r
    BF16 = mybir.dt.bfloat16
    AF = mybir.ActivationFunctionType
    OP = mybir.AluOpType

    # Keep every activation func we use inside one table
    # (natural_log_exp_and_others) so the chooser never thrashes
    # ACT_TABLE_LOADs between Exp and Ln.
    need = {AF.Exp, AF.Ln, AF.Square, AF.Copy, AF.Identity}
    keep = "natural_log_exp_and_others"
    tabs = hw_specs.get_activation_tables("gen3")
    if keep in tabs and need <= tabs[keep]:
        for name, s in tabs.items():
            if name != keep:
                s -= need

    nch = t // C              # chunks
    nts = t // 512            # 512-wide time slices
    assert t % 512 == 0

    nc = bacc.Bacc("TRN2", target_bir_lowering=False, debug=False)

    xt_d = nc.dram_tensor("xt", [128, 4, t], BF16, kind="ExternalInput")
    wqk_d = nc.dram_tensor("wqk", [128, 4, 2 * dk], BF16, kind="ExternalInput")
    wvg_d = nc.dram_tensor("wvg", [128, 4, 2 * dv], BF16, kind="ExternalInput")
    wgk_d = nc.dram_tensor("wgk2", [128, 4, 2 * dk], BF16,
                           kind="ExternalInput")
    wf_d = nc.dram_tensor("wfused", [dv, 10], BF16, kind="ExternalInput")
    nb_d = nc.dram_tensor("nbgk2", [128, 1], F32, kind="ExternalInput")
    um_d = nc.dram_tensor("umask4", [C, 4 * C], BF16, kind="ExternalInput")
    out_d = nc.dram_tensor("out10", [10, t], F32, kind="ExternalOutput")

    with tile.TileContext(nc) as tc:
        with (
            tc.tile_pool(name="wt", bufs=1) as wt,
            tc.tile_pool(name="big", bufs=1) as big,
            tc.tile_pool(name="sl", bufs=2) as sl,      # per-slice sbuf tiles
            tc.tile_pool(name="st", bufs=2) as st,      # S state (bf16)
            tc.tile_pool(name="pj", bufs=3, space="PSUM") as pj,  # projections
            tc.tile_pool(name="pp", bufs=2, space="PSUM") as pp,  # pat/poT/p10
            tc.tile_pool(name="pd", bufs=2, space="PSUM") as pd,  # pds
            tc.tile_pool(name="px", bufs=1, space="PSUM") as px,  # msT/bc
        ):
            # ---- weights / consts ----
            wqk_sb = wt.tile([128, 4, 2 * dk], BF16)
            wvg_sb = wt.tile([128, 4, 2 * dv], BF16)
            wgk_sb = wt.tile([128, 4, 2 * dk], BF16)
            wf_sb = wt.tile([dv, 10], BF16)
            nb_sb = wt.tile([128, 1], F32)
            um_sb = wt.tile([C, 4 * C], BF16)
            eps_sb = wt.tile([128, 1], F32)
            nc.vector.memset(eps_sb[:], EPS)
            ones_sb = wt.tile([128, 1], F32)
            nc.vector.memset(ones_sb[:], 1.0)
            onesb_sb = wt.tile([dv, 1], BF16)
            nc.vector.memset(onesb_sb[:], 1.0)
            ones1_sb = wt.tile([1, 10], BF16)
            nc.vector.memset(ones1_sb[:], 1.0)
            # eeqk factors: q-half exp(-spc/16 + ln(scale)), k-half
            # exp(+spc/16)
            scv_sb = wt.tile([128, 1], F32)
            nc.vector.memset(scv_sb[0:dk, :], -1.0 / GATE_NORM)
            nc.vector.memset(scv_sb[dk:128, :], 1.0 / GATE_NORM)
            bv_sb = wt.tile([128, 1], F32)
            nc.vector.memset(bv_sb[0:dk, :], float(np.log(SCALE)))
            nc.vector.memset(bv_sb[dk:128, :], 0.0)

            # scan reset mask: 0 at chunk starts, 1 elsewhere
            mres = wt.tile([128, 512], F32)
            nc.vector.memset(mres[:], 1.0)
            mres_v = mres[:].rearrange("p (c l) -> p c l", l=C)
            nc.vector.memset(mres_v[:, :, 0:1], 0.0)

            # ---- big SBUF tensors ----
            xT = big.tile([128, 4, t], BF16)      # x^T per 128-d-chunk
            sp = big.tile([128, t], F32)
            spc = big.tile([128, t], F32)
            qt = big.tile([dk, t], BF16)          # q-tilde (base partition 0)
            kt = big.tile([dk, t], BF16)          # k-tilde (base partition 0)
            kh = big.tile([dk, t], BF16)          # kt * dlast
            ktn = big.tile([C, nch, dk], BF16)    # kh^T per chunk
            vb = big.tile([C, nch, dv], BF16)     # v natural
            swT = big.tile([dv, t], BF16)         # swish(g)^T
            dlast = big.tile([128, nch], F32)
            rstdT = big.tile([1, t], BF16)
            out_sb = big.tile([10, t], F32)

            spc_v = spc[:].rearrange("p (c l) -> p c l", l=C)

            # ---- all input DMAs on the sync HWDGE queue (keeps the
            # scalar engine free for activations); gate weights + first x
            # pieces first so the first matmuls start early
            nc.sync.dma_start(wgk_sb[:], wgk_d[:])
            nc.sync.dma_start(nb_sb[:], nb_d[:])
            for i in range(4):
                nc.sync.dma_start(xT[:, :, i * C:(i + 1) * C],
                                  xt_d[:, :, i * C:(i + 1) * C])
            nc.sync.dma_start(wqk_sb[:], wqk_d[:])
            nc.sync.dma_start(wvg_sb[:], wvg_d[:])
            nc.sync.dma_start(um_sb[:], um_d[:])
            nc.sync.dma_start(wf_sb[:], wf_d[:])
            if nts > 1:
                nc.sync.dma_start(xT[:, :, 512:1024], xt_d[:, :, 512:1024])

            # ---- PE warm-up burst: dependency-free matmuls during input
            # staging keep the HAM activity window busy so the real matmuls
            # start at K=8/8 (2.4 GHz) instead of cold (1.2 GHz).
            wz = wt.tile([128, 512], BF16)
            nc.vector.memset(wz[:], 0.0)
            pwarm = px.tile([10, 512], F32, tag="X")
            for _ in range(26):
                nc.tensor.matmul(pwarm[:], wz[:, 0:10], wz[:],
                                 start=True, stop=True)

            def emit_proj(j):
                ts = slice(j * 512, (j + 1) * 512)
                # gate chain (duplicated on both partition halves):
                # z -> sp = ln(1+exp(-z-b)) -> masked-reset cumsum
                pg = pj.tile([128, 512], F32, tag="P")
                for d4 in range(4):
                    nc.tensor.matmul(pg[:], wgk_sb[:, d4, :], xT[:, d4, ts],
                                     start=(d4 == 0), stop=(d4 == 3))
                eg = sl.tile([128, 512], F32, tag="eg")
                nc.scalar.activation(out=eg[:], in_=pg[:], func=AF.Exp,
                                     scale=-1.0, bias=nb_sb[:])
                nc.scalar.activation(out=sp[:, ts], in_=eg[:], func=AF.Ln,
                                     bias=ones_sb[:])
                nc.vector.tensor_tensor_scan(
                    out=spc[:, ts], data0=mres[:], data1=sp[:, ts],
                    initial=0.0, op0=OP.mult, op1=OP.add)
                nc.scalar.activation(
                    out=dlast[:, 4 * j:4 * j + 4],
                    in_=spc_v[:, 4 * j:4 * j + 4, C - 1:C],
                    func=AF.Exp, scale=-1.0 / GATE_NORM)
                eeqk = sl.tile([128, 512], BF16, tag="ee")
                nc.scalar.activation(out=eeqk[:], in_=spc[:, ts], func=AF.Exp,
                                     scale=scv_sb[:], bias=bv_sb[:])

                # q|k projection (fp32r), decay applied on psum eviction
                pqk = pj.tile([128, 512], F32, tag="P")
                for d4 in range(4):
                    nc.tensor.matmul(pqk[:], wqk_sb[:, d4, :], xT[:, d4, ts],
                                     start=(d4 == 0), stop=(d4 == 3))
                nc.vector.tensor_mul(out=kt[:, ts], in0=pqk[dk:128, :],
                                     in1=eeqk[dk:128, :])
                nc.vector.tensor_mul(out=qt[:, ts], in0=pqk[0:dk, :],
                                     in1=eeqk[0:dk, :])

                # v / g projections, weight-stationary (transposed outputs)
                pv = pj.tile([dv, 512], F32, tag="P")
                for d4 in range(4):
                    nc.tensor.matmul(pv[:], wvg_sb[:, d4, 0:dv],
                                     xT[:, d4, ts],
                                     start=(d4 == 0), stop=(d4 == 3))
                pgt = pj.tile([dv, 512], F32, tag="P")
                for d4 in range(4):
                    nc.tensor.matmul(pgt[:], wvg_sb[:, d4, dv:2 * dv],
                                     xT[:, d4, ts],
                                     start=(d4 == 0), stop=(d4 == 3))
                vt = sl.tile([dv, 512], BF16, tag="vt")
                nc.scalar.activation(out=vt[:], in_=pv[:], func=AF.Copy)
                nc.sync.dma_start(vb[:, 4 * j:4 * j + 4, :], vt[:],
                                  transpose=True)
                if j + 2 < nts:
                    nx = slice((j + 2) * 512, (j + 3) * 512)
                    nc.sync.dma_start(xT[:, :, nx], xt_d[:, :, nx])
                e2 = sl.tile([dv, 512], BF16, tag="e2")
                nc.scalar.activation(out=e2[:], in_=pgt[:], func=AF.Exp,
                                     scale=-1.0)
                return pgt, e2

            def emit_kh(j):
                # state-scaled k rows + batched chunk-transpose; deferred so
                # the PE-critical atm/S-update DVE ops of the previous slice
                # run first (ktn is only needed one slice later)
                ts = slice(j * 512, (j + 1) * 512)
                for i in range(4):
                    tt = 4 * j + i
                    cs = slice(tt * C, (tt + 1) * C)
                    nc.vector.tensor_scalar_mul(
                        out=kh[:, cs], in0=kt[:, cs],
                        scalar1=dlast[0:dk, tt:tt + 1])
                nc.sync.dma_start(ktn[:, 4 * j:4 * j + 4, 0:dk],
                                  kh[:, ts], transpose=True)

            def emit_swish(j, pgt, e2):
                # deferred swish DVE chain so it never sits ahead of the
                # PE-critical atm/S-update DVE work in the vector queue
                ts = slice(j * 512, (j + 1) * 512)
                dn = sl.tile([dv, 512], F32, tag="dn")
                nc.vector.tensor_scalar_add(out=dn[:], in0=e2[:], scalar1=1.0)
                rc = sl.tile([dv, 512], F32, tag="rc")
                nc.vector.reciprocal_approx_fast(out=rc[:], in_=dn[:])
                nc.vector.tensor_mul(out=swT[:, ts], in0=pgt[:], in1=rc[:])

            S_prev = st.tile([dk, dv], BF16, tag="S")
            nc.vector.memset(S_prev[:], 0.0)

            def emit_core(j):
                nonlocal S_prev
                # intra-chunk attention matrices, batched mask+evict
                pat = pp.tile([C, 4, C], F32, tag="O")
                for i in range(4):
                    tt = 4 * j + i
                    cs = slice(tt * C, (tt + 1) * C)
                    nc.tensor.matmul(pat[:, i, :], kt[:, cs],
                                     qt[:, cs], start=True, stop=True)
                atm = sl.tile([C, 4, C], BF16, tag="atm")
                nc.vector.tensor_mul(out=atm[:], in0=pat[:], in1=um_sb[:])

                # state updates: pds pre-scaled by dlast via kh
                pds_l = []
                for i in range(4):
                    tt = 4 * j + i
                    pds = pd.tile([dk, dv], F32, tag="D")
                    nc.tensor.matmul(pds[:], ktn[:, tt, :], vb[:, tt, :],
                                     start=True, stop=True)
                    pds_l.append(pds)
                S_l = []
                for i in range(4):
                    tt = 4 * j + i
                    S_new = st.tile([dk, dv], BF16, tag="S")
                    nc.vector.scalar_tensor_tensor(
                        out=S_new[:], in0=S_prev[:],
                        scalar=dlast[0:dk, tt:tt + 1], in1=pds_l[i][:],
                        op0=OP.mult, op1=OP.add)
                    S_l.append(S_prev)
                    S_prev = S_new

                # transposed outputs: poT[e,t] = vb_c^T atm + S^T qt
                poT = pp.tile([dv, 4, C], F32, tag="O")
                for i in range(4):
                    tt = 4 * j + i
                    cs = slice(tt * C, (tt + 1) * C)
                    nc.tensor.matmul(poT[:, i, :], vb[:, tt, :], atm[:, i, :],
                                     start=True, stop=False)
                    nc.tensor.matmul(poT[:, i, :], S_l[i][:],
                                     qt[:, cs], start=False, stop=True)
                sqT = sl.tile([dv, 4, C], BF16, tag="sqT")
                nc.scalar.activation(out=sqT[:], in_=poT[:], func=AF.Square)
                return poT, sqT

            def emit_stats(j, sqT):
                ts = slice(j * 512, (j + 1) * 512)
                msT = px.tile([1, 512], F32, tag="X")
                nc.tensor.matmul(msT[:], onesb_sb[:],
                                 sqT[:].rearrange("p c l -> p (c l)"),
                                 start=True, stop=True)
                lnvT = sl.tile([1, 512], F32, tag="lnvT")
                nc.scalar.activation(out=lnvT[:], in_=msT[:], func=AF.Ln,
                                     scale=1.0 / dv, bias=eps_sb[0:1, :])
                nc.scalar.activation(out=rstdT[:, ts], in_=lnvT[:],
                                     func=AF.Exp, scale=-0.5)

            def emit_og(j, poT):
                ts = slice(j * 512, (j + 1) * 512)
                og = sl.tile([dv, 512], BF16, tag="og")
                nc.vector.tensor_mul(
                    out=og[:], in0=poT[:].rearrange("p c l -> p (c l)"),
                    in1=swT[:, ts])
                return og

            def emit_tail(j, og):
                ts = slice(j * 512, (j + 1) * 512)
                bc = pd.tile([10, 512], F32, tag="D")
                nc.tensor.matmul(bc[:], ones1_sb[:], rstdT[:, ts],
                                 start=True, stop=True)
                bcs = sl.tile([10, 512], F32, tag="bcs")
                nc.scalar.activation(out=bcs[:], in_=bc[:], func=AF.Copy)
                p10 = pp.tile([10, 512], F32, tag="O")
                nc.tensor.matmul(p10[:], wf_sb[:], og[:],
                                 start=True, stop=True)
                nc.vector.tensor_mul(out=out_sb[:, ts], in0=p10[:],
                                     in1=bcs[:])
                nc.sync.dma_start(out_d[:, ts], out_sb[:, ts])

            sw_args = {}
            po_args = {}
            og_args = {}
            for j in range(nts):
                sw_args[j] = emit_proj(j)
                if j > 0:
                    po_args[j - 1], sq = emit_core(j - 1)
                    if j > 1:
                        emit_tail(j - 2, og_args.pop(j - 2))
                    emit_stats(j - 1, sq)
                emit_kh(j)
                if j > 0:
                    og_args[j - 1] = emit_og(j - 1, po_args.pop(j - 1))
                emit_swish(j, *sw_args.pop(j))
            po_args[nts - 1], sq = emit_core(nts - 1)
            if nts > 1:
                emit_tail(nts - 2, og_args.pop(nts - 2))
            emit_stats(nts - 1, sq)
            og_args[nts - 1] = emit_og(nts - 1, po_args.pop(nts - 1))
            emit_tail(nts - 1, og_args.pop(nts - 1))

    nc.compile()
    return nc


def _prep_inputs(inputs, t=T):
    """Per-core input dicts: core = 4*b + h."""
    ins = {k: np.ascontiguousarray(np.asarray(v, dtype=np.float32))
           for k, v in inputs.items()}
    x, Wq, Wk, Wv, Wg = ins["x"], ins["Wq"], ins["Wk"], ins["Wv"], ins["Wg"]
    Wgk12 = (ins["Wgk1"].astype(np.float64) @ ins["Wgk2"].astype(np.float64))
    bgk2, gnorm = ins["bgk2"], ins["gnorm_w"]
    Wo, Whead = ins["Wo"], ins["Whead"]

    um = (np.arange(C)[:, None] <= np.arange(C)[None, :]).astype(np.float32)
    um4 = np.ascontiguousarray(
        np.tile(um, (1, 4)).astype(ml_dtypes.bfloat16))

    def chunk_w(w):  # [512, n] -> [128, 4, n]
        return np.ascontiguousarray(
            w.reshape(4, 128, -1).transpose(1, 0, 2).astype(
                ml_dtypes.bfloat16))

    in_maps = []
    for core in range(8):
        b, h = divmod(core, 4)
        wf = ((gnorm[:, None].astype(np.float64)
               * Wo[h * dv:(h + 1) * dv, :].astype(np.float64))
              @ Whead.astype(np.float64)).astype(ml_dtypes.bfloat16)
        wgk_h = Wgk12[:, h * dk:(h + 1) * dk].astype(np.float32)
        nb_h = -bgk2[h * dk:(h + 1) * dk, None]
        in_maps.append({
            "xt": np.ascontiguousarray(
                x[b, :t].T.reshape(4, 128, t).transpose(1, 0, 2).astype(
                    ml_dtypes.bfloat16)),
            "wqk": chunk_w(np.concatenate(
                [Wq[:, h * dk:(h + 1) * dk], Wk[:, h * dk:(h + 1) * dk]], 1)),
            "wvg": chunk_w(np.concatenate(
                [Wv[:, h * dv:(h + 1) * dv], Wg[:, h * dv:(h + 1) * dv]], 1)),
            "wgk2": chunk_w(np.concatenate([wgk_h, wgk_h], 1)),
            "wfused": np.ascontiguousarray(wf),
            "nbgk2": np.ascontiguousarray(
                np.concatenate([nb_h, nb_h], 0).astype(np.float32)),
            "umask4": um4,
        })
    return in_maps


def _gather(results, inputs, t=T):
    bhead = np.asarray(inputs["bhead"], dtype=np.float32)
    out = np.zeros((B, t, 10), np.float32)
    for core in range(8):
        b = core // 4
        out[b] += results[core]["out10"].T
    out += bhead[None, None, :]
    return out


def run(inputs, trace=False, **kw):
    from concourse.bass_utils import run_bass_kernel_spmd
    if "nc" not in _CACHE:
        _CACHE["nc"] = build()
    nc = _CACHE["nc"]
    in_maps = _prep_inputs(inputs)
    res = run_bass_kernel_spmd(nc, in_maps, core_ids=list(range(8)),
                               trace=trace, **kw)
    return _gather(res.results, inputs), res


def kernel(**inputs) -> np.ndarray:
    out, _ = run(inputs, trace=False)
    return out


# revision 23
# speedup vs baseline: 1.1500x; 1.1500x over previous
"""Gated linear attention (GLA) Bass kernel for Trainium2, 8 NeuronCores.

Sharding: one core per (batch, head) pair -- B=2 x H=4 = 8 cores.
Each core computes its head's full pipeline with a chunked-parallel form of
the gated recurrence (chunk = 128), entirely on-device.

v3 structure (per 512-wide time slice j):
  gate:  pg = [Wgk|Wgk]^T x^T (duplicated M=128) -> sp = softplus ->
         per-chunk cumsum (all on [128,512] with both halves identical)
         -> eeqk = exp(scv*spc + bv) one ACT op (scv/bv per-partition
         vectors: q-half -G/16 + ln(scale), k-half +G/16)
         -> dlast = exp(-spc_last/16) on all 128 rows
  qk:    pqk = [Wq|Wk]^T x^T -> qkt = pqk * eeqk (one DVE op, bf16)
         kh[64:] = qkt[64:] * dlast ; ktn = kh^T, ONE batched DMA-transpose
  vg:    pv = Wv^T x^T, pgt = Wg^T x^T (transposed [dv,512])
         vb = v natural via ONE batched DMA-transpose of the pv eviction
         swT = pgt * 1/(1+exp(-pgt)) (ACT exp + DVE add/recip/mul)
  chunks (bf16 matmuls):
         pat[s,t] = kt_c^T qt_c ; atm = mask(pat) (batched per slice)
         poT = Wv-side: poT[e,t] = vb_c^T atm + S^T qt_c  (transposed O!)
         pds = ktn^T vb (pre-scaled by dlast via kh)
         S' = S*dlast + pds  (single fused scalar_tensor_tensor)
  out (no transposes):
         onbT = bf16(poT); sqT = poT^2 ; msT[1,512] = ones^T sqT (matmul)
         rstdT = exp(-.5 ln(ms/dv+eps)); bc[10,512] = ones1^T rstdT (matmul)
         og = onbT * swT ; p10 = wf^T og ; out = p10 * bc (rstd folded in)

Host does weight-only preprocessing (Wgk1@Wgk2, gnorm*Wo@Whead) and the
final head-sum + bhead add.
"""
import sys, os
sys.path.insert(0, "/opt/trn_rl_repo")

import numpy as np
import ml_dtypes

B, T, D = 2, 2048, 512
H = 4
dk, dv = 64, 128          # per-head key/value dims
C = 128                   # chunk length
GATE_NORM = 16.0
EPS = 1e-5
SCALE = dk ** -0.5

_CACHE = {}


def build(t=T):
    import concourse.bass as bass  # noqa: F401
    from concourse import bacc, mybir
    import concourse.tile as tile
    import concourse.hw_specs as hw_specs

    F32 = mybir.dt.float32
    F32R = mybir.dt.float32r
    BF16 = mybir.dt.bfloat16
    AF = mybir.ActivationFunctionType
    OP = mybir.AluOpType

    # Keep every activation func we use inside one table
    # (natural_log_exp_and_others) so the chooser never thrashes
    # ACT_TABLE_LOADs between Exp and Ln.
    need = {AF.Exp, AF.Ln, AF.Square, AF.Copy, AF.Identity}
    keep = "natural_log_exp_and_others"
    tabs = hw_specs.get_activation_tables("gen3")
    if keep in tabs and need <= tabs[keep]:
        for name, s in tabs.items():
            if name != keep:
                s -= need

    nch = t // C              # chunks
    nts = t // 512            # 512-wide time slices
    assert t % 512 == 0

    nc = bacc.Bacc("TRN2", target_bir_lowering=False, debug=False)

    xt_d = nc.dram_tensor("xt", [128, 4, t], BF16, kind="ExternalInput")
    wqk_d = nc.dram_tensor("wqk", [128, 4, 2 * dk], BF16, kind="ExternalInput")
    wvg_d = nc.dram_tensor("wvg", [128, 4, 2 * dv], BF16, kind="ExternalInput")
    wgk_d = nc.dram_tensor("wgk2", [128, 4, 2 * dk], BF16,
                           kind="ExternalInput")
    wf_d = nc.dram_tensor("wfused", [dv, 10], BF16, kind="ExternalInput")
    nb_d = nc.dram_tensor("nbgk2", [128, 1], F32, kind="ExternalInput")
    um_d = nc.dram_tensor("umask4", [C, 4 * C], BF16, kind="ExternalInput")
    out_d = nc.dram_tensor("out10", [10, t], F32, kind="ExternalOutput")

    with tile.TileContext(nc) as tc:
        with (
            tc.tile_pool(name="wt", bufs=1) as wt,
            tc.tile_pool(name="big", bufs=1) as big,
            tc.tile_pool(name="sl", bufs=2) as sl,      # per-slice sbuf tiles
            tc.tile_pool(name="st", bufs=2) as st,      # S state (bf16)
            tc.tile_pool(name="pj", bufs=3, space="PSUM") as pj,  # projections
            tc.tile_pool(name="pp", bufs=2, space="PSUM") as pp,  # pat/poT/p10
            tc.tile_pool(name="pd", bufs=2, space="PSUM") as pd,  # pds
            tc.tile_pool(name="px", bufs=1, space="PSUM") as px,  # msT/bc
        ):
            # ---- weights / consts ----
            wqk_sb = wt.tile([128, 4, 2 * dk], BF16)
            wvg_sb = wt.tile([128, 4, 2 * dv], BF16)
            wgk_sb = wt.tile([128, 4, 2 * dk], BF16)
            wf_sb = wt.tile([dv, 10], BF16)
            nb_sb = wt.tile([128, 1], F32)
            um_sb = wt.tile([C, 4 * C], BF16)
            eps_sb = wt.tile([128, 1], F32)
            nc.vector.memset(eps_sb[:], EPS)
            ones_sb = wt.tile([128, 1], F32)
            nc.vector.memset(ones_sb[:], 1.0)
            onesb_sb = wt.tile([dv, 1], BF16)
            nc.vector.memset(onesb_sb[:], 1.0)
            ones1_sb = wt.tile([1, 10], BF16)
            nc.vector.memset(ones1_sb[:], 1.0)
            # eeqk factors: q-half exp(-spc/16 + ln(scale)), k-half
            # exp(+spc/16)
            scv_sb = wt.tile([128, 1], F32)
            nc.vector.memset(scv_sb[0:dk, :], -1.0 / GATE_NORM)
            nc.vector.memset(scv_sb[dk:128, :], 1.0 / GATE_NORM)
            bv_sb = wt.tile([128, 1], F32)
            nc.vector.memset(bv_sb[0:dk, :], float(np.log(SCALE)))
            nc.vector.memset(bv_sb[dk:128, :], 0.0)

            # scan reset mask: 0 at chunk starts, 1 elsewhere
            mres = wt.tile([128, 512], F32)
            nc.vector.memset(mres[:], 1.0)
            mres_v = mres[:].rearrange("p (c l) -> p c l", l=C)
            nc.vector.memset(mres_v[:, :, 0:1], 0.0)

            # ---- big SBUF tensors ----
            xT = big.tile([128, 4, t], BF16)      # x^T per 128-d-chunk
            sp = big.tile([128, t], F32)
            spc = big.tile([128, t], F32)
            qt = big.tile([dk, t], BF16)          # q-tilde (base partition 0)
            kt = big.tile([dk, t], BF16)          # k-tilde (base partition 0)
            kh = big.tile([dk, t], BF16)          # kt * dlast
            ktn = big.tile([C, nch, dk], BF16)    # kh^T per chunk
            vb = big.tile([C, nch, dv], BF16)     # v natural
            swT = big.tile([dv, t], BF16)         # swish(g)^T
            dlast = big.tile([128, nch], F32)
            rstdT = big.tile([1, t], BF16)
            out_sb = big.tile([10, t], F32)

            spc_v = spc[:].rearrange("p (c l) -> p c l", l=C)

            # ---- all input DMAs on the sync HWDGE queue (keeps the
            # scalar engine free for activations); gate weights + first x
            # pieces first so the first matmuls start early
            nc.sync.dma_start(wgk_sb[:], wgk_d[:])
            nc.sync.dma_start(nb_sb[:], nb_d[:])
            for i in range(4):
                nc.sync.dma_start(xT[:, :, i * C:(i + 1) * C],
                                  xt_d[:, :, i * C:(i + 1) * C])
            nc.sync.dma_start(wqk_sb[:], wqk_d[:])
            nc.sync.dma_start(wvg_sb[:], wvg_d[:])
            nc.sync.dma_start(um_sb[:], um_d[:])
            nc.sync.dma_start(wf_sb[:], wf_d[:])
            if nts > 1:
                nc.sync.dma_start(xT[:, :, 512:1024], xt_d[:, :, 512:1024])

            # ---- PE warm-up burst: dependency-free matmuls during input
            # staging keep the HAM activity window busy so the real matmuls
            # start at K=8/8 (2.4 GHz) instead of cold (1.2 GHz).
            wz = wt.tile([128, 512], BF16)
            nc.vector.memset(wz[:], 0.0)
            pwarm = px.tile([10, 512], F32, tag="X")
            for _ in range(26):
                nc.tensor.matmul(pwarm[:], wz[:, 0:10], wz[:],
                                 start=True, stop=True)

            def emit_proj(j):
                ts = slice(j * 512, (j + 1) * 512)
                # gate chain (duplicated on both partition halves):
                # z -> sp = ln(1+exp(-z-b)) -> masked-reset cumsum
                pg = pj.tile([128, 512], F32, tag="P")
                for d4 in range(4):
                    nc.tensor.matmul(pg[:], wgk_sb[:, d4, :], xT[:, d4, ts],
                                     start=(d4 == 0), stop=(d4 == 3))
                eg = sl.tile([128, 512], F32, tag="eg")
                nc.scalar.activation(out=eg[:], in_=pg[:], func=AF.Exp,
                                     scale=-1.0, bias=nb_sb[:])
                nc.scalar.activation(out=sp[:, ts], in_=eg[:], func=AF.Ln,
                                     bias=ones_sb[:])
                nc.vector.tensor_tensor_scan(
                    out=spc[:, ts], data0=mres[:], data1=sp[:, ts],
                    initial=0.0, op0=OP.mult, op1=OP.add)
                nc.scalar.activation(
                    out=dlast[:, 4 * j:4 * j + 4],
                    in_=spc_v[:, 4 * j:4 * j + 4, C - 1:C],
                    func=AF.Exp, scale=-1.0 / GATE_NORM)
                eeqk = sl.tile([128, 512], BF16, tag="ee")
                nc.scalar.activation(out=eeqk[:], in_=spc[:, ts], func=AF.Exp,
                                     scale=scv_sb[:], bias=bv_sb[:])

                # q|k projection (fp32r), decay applied on psum eviction
                pqk = pj.tile([128, 512], F32, tag="P")
                for d4 in range(4):
                    nc.tensor.matmul(pqk[:], wqk_sb[:, d4, :], xT[:, d4, ts],
                                     start=(d4 == 0), stop=(d4 == 3))
                nc.vector.tensor_mul(out=kt[:, ts], in0=pqk[dk:128, :],
                                     in1=eeqk[dk:128, :])
                nc.vector.tensor_mul(out=qt[:, ts], in0=pqk[0:dk, :],
                                     in1=eeqk[0:dk, :])

                # v / g projections, weight-stationary (transposed outputs)
                pv = pj.tile([dv, 512], F32, tag="P")
                for d4 in range(4):
                    nc.tensor.matmul(pv[:], wvg_sb[:, d4, 0:dv],
                                     xT[:, d4, ts],
                                     start=(d4 == 0), stop=(d4 == 3))
                pgt = pj.tile([dv, 512], F32, tag="P")
                for d4 in range(4):
                    nc.tensor.matmul(pgt[:], wvg_sb[:, d4, dv:2 * dv],
                                     xT[:, d4, ts],
                                     start=(d4 == 0), stop=(d4 == 3))
                vt = sl.tile([dv, 512], BF16, tag="vt")
                nc.scalar.activation(out=vt[:], in_=pv[:], func=AF.Copy)
                nc.sync.dma_start(vb[:, 4 * j:4 * j + 4, :], vt[:],
                                  transpose=True)
                if j + 2 < nts:
                    nx = slice((j + 2) * 512, (j + 3) * 512)
                    nc.sync.dma_start(xT[:, :, nx], xt_d[:, :, nx])
                e2 = sl.tile([dv, 512], BF16, tag="e2")
                nc.scalar.activation(out=e2[:], in_=pgt[:], func=AF.Exp,
                                     scale=-1.0)
                return pgt, e2

            def emit_kh(j):
                # state-scaled k rows + batched chunk-transpose; deferred so
                # the PE-critical atm/S-update DVE ops of the previous slice
                # run first (ktn is only needed one slice later)
                ts = slice(j * 512, (j + 1) * 512)
                for i in range(4):
                    tt = 4 * j + i
                    cs = slice(tt * C, (tt + 1) * C)
                    nc.vector.tensor_scalar_mul(
                        out=kh[:, cs], in0=kt[:, cs],
                        scalar1=dlast[0:dk, tt:tt + 1])
                nc.sync.dma_start(ktn[:, 4 * j:4 * j + 4, 0:dk],
                                  kh[:, ts], transpose=True)

            def emit_swish(j, pgt, e2):
                # deferred swish DVE chain so it never sits ahead of the
                # PE-critical atm/S-update DVE work in the vector queue
                ts = slice(j * 512, (j + 1) * 512)
                dn = sl.tile([dv, 512], F32, tag="dn")
                nc.vector.tensor_scalar_add(out=dn[:], in0=e2[:], scalar1=1.0)
                rc = sl.tile([dv, 512], F32, tag="rc")
                nc.vector.reciprocal_approx_fast(out=rc[:], in_=dn[:])
                nc.vector.tensor_mul(out=swT[:, ts], in0=pgt[:], in1=rc[:])

            S_prev = st.tile([dk, dv], BF16, tag="S")
            nc.vector.memset(S_prev[:], 0.0)

            def emit_core(j):
                nonlocal S_prev
                # intra-chunk attention matrices, batched mask+evict
                pat = pp.tile([C, 4, C], F32, tag="O")
                for i in range(4):
                    tt = 4 * j + i
                    cs = slice(tt * C, (tt + 1) * C)
                    nc.tensor.matmul(pat[:, i, :], kt[:, cs],
                                     qt[:, cs], start=True, stop=True)
                atm = sl.tile([C, 4, C], BF16, tag="atm")
                nc.vector.tensor_mul(out=atm[:], in0=pat[:], in1=um_sb[:])

                # state updates: pds pre-scaled by dlast via kh
                pds_l = []
                for i in range(4):
                    tt = 4 * j + i
                    pds = pd.tile([dk, dv], F32, tag="D")
                    nc.tensor.matmul(pds[:], ktn[:, tt, :], vb[:, tt, :],
                                     start=True, stop=True)
                    pds_l.append(pds)
                S_l = []
                for i in range(4):
                    tt = 4 * j + i
                    S_new = st.tile([dk, dv], BF16, tag="S")
                    nc.vector.scalar_tensor_tensor(
                        out=S_new[:], in0=S_prev[:],
                        scalar=dlast[0:dk, tt:tt + 1], in1=pds_l[i][:],
                        op0=OP.mult, op1=OP.add)
                    S_l.append(S_prev)
                    S_prev = S_new

                # transposed outputs: poT[e,t] = vb_c^T atm + S^T qt
                poT = pp.tile([dv, 4, C], F32, tag="O")
                for i in range(4):
                    tt = 4 * j + i
                    cs = slice(tt * C, (tt + 1) * C)
                    nc.tensor.matmul(poT[:, i, :], vb[:, tt, :], atm[:, i, :],
                                     start=True, stop=False)
                    nc.tensor.matmul(poT[:, i, :], S_l[i][:],
                                     qt[:, cs], start=False, stop=True)
                ts = slice(j * 512, (j + 1) * 512)
                sqT = sl.tile([dv, 4, C], BF16, tag="sqT")
                nc.scalar.activation(out=sqT[:], in_=poT[:], func=AF.Square)
                # stats stage immediately: msT matmul + lnv + rstd land
                # ahead of the next slice's ACT work so the later bc/p10
                # matmuls never stall the PE long enough to re-throttle HAM
                msT = px.tile([1, 512], F32, tag="X")
                nc.tensor.matmul(msT[:], onesb_sb[:],
                                 sqT[:].rearrange("p c l -> p (c l)"),
                                 start=True, stop=True)
                lnvT = sl.tile([1, 512], F32, tag="lnvT")
                nc.scalar.activation(out=lnvT[:], in_=msT[:], func=AF.Ln,
                                     scale=1.0 / dv, bias=eps_sb[0:1, :])
                nc.scalar.activation(out=rstdT[:, ts], in_=lnvT[:],
                                     func=AF.Exp, scale=-0.5)
                return poT

            def emit_og(j, poT):
                ts = slice(j * 512, (j + 1) * 512)
                og = sl.tile([dv, 512], BF16, tag="og")
                nc.vector.tensor_mul(
                    out=og[:], in0=poT[:].rearrange("p c l -> p (c l)"),
                    in1=swT[:, ts])
                return og

            def emit_tail(j, og):
                ts = slice(j * 512, (j + 1) * 512)
                bc = px.tile([10, 512], F32, tag="X")
                nc.tensor.matmul(bc[:], ones1_sb[:], rstdT[:, ts],
                                 start=True, stop=True)
                bcs = sl.tile([10, 512], F32, tag="bcs")
                nc.scalar.activation(out=bcs[:], in_=bc[:], func=AF.Copy)
                p10 = pp.tile([10, 512], F32, tag="O")
                nc.tensor.matmul(p10[:], wf_sb[:], og[:],
                                 start=True, stop=True)
                nc.vector.tensor_mul(out=out_sb[:, ts], in0=p10[:],
                                     in1=bcs[:])
                nc.sync.dma_start(out_d[:, ts], out_sb[:, ts])

            sw_args = {}
            po_args = {}
            og_args = {}
            for j in range(nts):
                sw_args[j] = emit_proj(j)
                if j > 0:
                    po_args[j - 1] = emit_core(j - 1)
                emit_kh(j)
                if j > 0:
                    og_args[j - 1] = emit_og(j - 1, po_args.pop(j - 1))
                emit_swish(j, *sw_args.pop(j))
                if j > 1:
                    emit_tail(j - 2, og_args.pop(j - 2))
            po_args[nts - 1] = emit_core(nts - 1)
            og_args[nts - 1] = emit_og(nts - 1, po_args.pop(nts - 1))
            for j in sorted(og_args):
                emit_tail(j, og_args[j])
            og_args.clear()

    nc.compile()
    return nc


def _prep_inputs(inputs, t=T):
    """Per-core input dicts: core = 4*b + h."""
    ins = {k: np.ascontiguousarray(np.asarray(v, dtype=np.float32))
           for k, v in inputs.items()}
    x, Wq, Wk, Wv, Wg = ins["x"], ins["Wq"], ins["Wk"], ins["Wv"], ins["Wg"]
    Wgk12 = (ins["Wgk1"].astype(np.float64) @ ins["Wgk2"].astype(np.float64))
    bgk2, gnorm = ins["bgk2"], ins["gnorm_w"]
    Wo, Whead = ins["Wo"], ins["Whead"]

    um = (np.arange(C)[:, None] <= np.arange(C)[None, :]).astype(np.float32)
    um4 = np.ascontiguousarray(
        np.tile(um, (1, 4)).astype(ml_dtypes.bfloat16))

    def chunk_w(w):  # [512, n] -> [128, 4, n]
        return np.ascontiguousarray(
            w.reshape(4, 128, -1).transpose(1, 0, 2).astype(
                ml_dtypes.bfloat16))

    in_maps = []
    for core in range(8):
        b, h = divmod(core, 4)
        wf = ((gnorm[:, None].astype(np.float64)
               * Wo[h * dv:(h + 1) * dv, :].astype(np.float64))
              @ Whead.astype(np.float64)).astype(ml_dtypes.bfloat16)
        wgk_h = Wgk12[:, h * dk:(h + 1) * dk].astype(np.float32)
        nb_h = -bgk2[h * dk:(h + 1) * dk, None]
        in_maps.append({
            "xt": np.ascontiguousarray(
                x[b, :t].T.reshape(4, 128, t).transpose(1, 0, 2).astype(
                    ml_dtypes.bfloat16)),
            "wqk": chunk_w(np.concatenate(
                [Wq[:, h * dk:(h + 1) * dk], Wk[:, h * dk:(h + 1) * dk]], 1)),
            "wvg": chunk_w(np.concatenate(
                [Wv[:, h * dv:(h + 1) * dv], Wg[:, h * dv:(h + 1) * dv]], 1)),
            "wgk2": chunk_w(np.concatenate([wgk_h, wgk_h], 1)),
            "wfused": np.ascontiguousarray(wf),
            "nbgk2": np.ascontiguousarray(
                np.concatenate([nb_h, nb_h], 0).astype(np.float32)),
            "umask4": um4,
        })
    return in_maps


def _gather(results, inputs, t=T):
    bhead = np.asarray(inputs["bhead"], dtype=np.float32)
    out = np.zeros((B, t, 10), np.float32)
    for core in range(8):
        b = core // 4
        out[b] += results[core]["out10"].T
    out += bhead[None, None, :]
    return out


def run(inputs, trace=False, **kw):
    from concourse.bass_utils import run_bass_kernel_spmd
    if "nc" not in _CACHE:
        _CACHE["nc"] = build()
    nc = _CACHE["nc"]
    in_maps = _prep_inputs(inputs)
    res = run_bass_kernel_spmd(nc, in_maps, core_ids=list(range(8)),
                               trace=trace, **kw)
    return _gather(res.results, inputs), res


def kernel(**inputs) -> np.ndarray:
    out, _ = run(inputs, trace=False)
    return out
